# revision 12
# baseline (speedup 1.0000x reference)
"""Trainium2 Bass kernel for vLLM-style sampler (nn_Sampler_23897198035285).

Strategy
--------
The reference does, per row (N=256, V=128000):
  raw logprobs (log_softmax of logits), penalty application (<=96 touched
  positions/row), temperature, top-k (k<=63) / top-p filtering on a full
  descending sort, Gumbel-max sampling, and top-20 logprob gathering.

Everything downstream of the full-vocab scan only ever looks at the extreme
top of each row:
  * tokens surviving top-k have penalized-rank < k <= 63;
  * penalties only *decrease* values and touch <= 96 positions, so any token
    in the penalized top-64 has raw rank <= 64+96 = 160;
  * the sampled token's rank among raw logprobs is likewise <= 160.

So the device only needs two streaming quantities per row:
  1. sum(exp(logits)) for the log-softmax normalizer, and
  2. a candidate set provably containing the raw top-~160:
     the top-8 of each 500-element chunk (256 chunks/row -> 2048 candidates,
     with the max over per-chunk 8th-maxes giving an exact upper bound B on
     every non-candidate value).

Rows are sharded across the 8 cores (32 rows/core, zero communication).
Each core streams its 16.4MB logits shard once: DVE max/max_index extracts
candidates, ACT computes exp with a fused per-partition accumulate.
The host then finishes each row exactly on <=2048 candidates, verifying
coverage via B; rows that fail verification (statistically ~never) fall back
to exact host candidate extraction. `gumbel` is only ever read at the <=63
kept positions per row, so it is never shipped to the device at all.
"""

import numpy as np

N, V = 256, 128000
NCORES = 8
RPC = N // NCORES          # rows per core
P = 128                    # SBUF partitions
F = V // P                 # 1000 elements per partition per row
CH = 2                     # chunks per partition
CF = F // CH               # 500-element chunk
K8 = 8                     # top-k extracted per chunk (HW max8 width)
CAND = P * CH * K8         # 2048 candidates per row
_EPS = 1e-5

# test.py pokes these for profiling; the harness never touches them.
TRACE = False
LAST = {}

_prog = None


def _build_program():
    import concourse.mybir as mybir
    import concourse.tile as tile
    from concourse import bacc

    nc = bacc.Bacc("TRN2", target_bir_lowering=False, debug=False,
                   enable_asserts=False)
    f32 = mybir.dt.float32
    logits = nc.dram_tensor("logits", [RPC, V], f32, kind="ExternalInput")
    cand_idx = nc.dram_tensor("cand_idx", [P, RPC * CH * K8], mybir.dt.uint16,
                              kind="ExternalOutput")
    sumexp = nc.dram_tensor("sumexp", [P, RPC], f32, kind="ExternalOutput")

    with tile.TileContext(nc) as tc:
        with (
            tc.tile_pool(name="inp", bufs=6) as inp,
            tc.tile_pool(name="outp", bufs=1) as outp,
        ):
            vals_t = outp.tile([P, RPC * CH * K8], mybir.dt.bfloat16, tag="vals")
            idx_t = outp.tile([P, RPC * CH * K8], mybir.dt.uint16, tag="idx")
            se_t = outp.tile([P, RPC], f32, tag="se")
            # Per-row disjoint bf16 slices for the exp output. exp is
            # monotone, so the DVE top-8 extraction runs on the exp values
            # (the raw tile then has exactly one consumer): every
            # instruction here carries at most one semaphore wait, which is
            # all the ACT/DMA ISA structs have room for.
            e = outp.tile([P, RPC * F], mybir.dt.bfloat16, tag="e")

            lg = logits.ap().rearrange("r (p f) -> r p f", p=P)
            for r in range(RPC):
                t = inp.tile([P, F], f32, tag="t")
                nc.sync.dma_start(t[:], lg[r])
                er = e[:, r * F:(r + 1) * F]
                nc.scalar.activation(er, t[:],
                                     mybir.ActivationFunctionType.Exp,
                                     accum_out=se_t[:, r:r + 1])
                for c in range(CH):
                    sl = er[:, c * CF:(c + 1) * CF]
                    base = (r * CH + c) * K8
                    vs = vals_t[:, base:base + K8]
                    nc.vector.max(vs, sl)
                    nc.vector.max_index(idx_t[:, base:base + K8], vs, sl)
            nc.sync.dma_start(cand_idx.ap(), idx_t[:])
            nc.sync.dma_start(sumexp.ap(), se_t[:])
    nc.compile()
    return nc


def _run_device(logits):
    global _prog
    if _prog is None:
        _prog = _build_program()
    from concourse.bass_utils import run_bass_kernel_spmd

    in_maps = [
        {"logits": np.ascontiguousarray(logits[c * RPC:(c + 1) * RPC])}
        for c in range(NCORES)
    ]
    out = run_bass_kernel_spmd(_prog, in_maps, core_ids=list(range(NCORES)),
                               trace=TRACE)
    LAST["exec_time_ns"] = out.exec_time_ns
    rs = out.results

    ci = np.stack([r["cand_idx"] for r in rs])           # [8,128,RPC*16] u16
    se = np.stack([r["sumexp"] for r in rs])             # [8,128,RPC]

    ci = ci.reshape(NCORES, P, RPC, CH * K8).transpose(0, 2, 1, 3)
    ci = ci.reshape(N, P, CH, K8).astype(np.int64)

    ids = (np.arange(P, dtype=np.int64)[None, :, None, None] * F
           + np.arange(CH, dtype=np.int64)[None, None, :, None] * CF
           + np.minimum(ci, CF - 1))

    bad = (ci >= CF).reshape(N, -1).any(axis=1)          # max_index anomaly

    # exact f32 candidate values, gathered from the input itself
    vals = logits[np.arange(N)[:, None], ids.reshape(N, CAND)]
    ids = ids.reshape(N, CAND)

    # The device extracted the per-chunk top-8 *of the bf16 exp image*.
    # Every non-extracted element of a chunk maps to a bf16-exp bucket <=
    # the smallest extracted bucket, so its raw value is below the smallest
    # extracted raw value plus one bucket width (~0.004 in log space) plus
    # exp-table slack; 0.02 is generous for both.
    v4 = vals.reshape(N, P, CH, K8)
    bound = (v4.min(axis=3).max(axis=(1, 2)) + np.float32(0.02)).astype(np.float32)

    sid_sorted = np.sort(ids, axis=1)
    bad |= (sid_sorted[:, 1:] == sid_sorted[:, :-1]).any(axis=1)
    bad |= ~(bound > 0)

    sum0 = se.transpose(0, 2, 1).reshape(N, P).sum(axis=1, dtype=np.float64)
    return vals, ids, bound, sum0, bad


def _sort_desc(vals, ids):
    """Descending by value, ties by ascending id (matches stable argsort(-x))."""
    o = np.lexsort((ids, -vals), axis=-1)
    return (np.take_along_axis(vals, o, axis=-1),
            np.take_along_axis(ids, o, axis=-1))


def _penalized(svals, pm_c, oc_c, rep, freq, pres, temp_eff):
    """f32 mimicry of the reference penalty/temperature math, per candidate.

    svals: candidate raw logits (f32), pm_c: prompt-seen mask, oc_c: output
    counts (f32), penalties broadcastable to svals.
    """
    out_mask = oc_c > 0
    seen = pm_c | out_mask
    x = np.where(seen, np.where(svals > 0, svals / rep, svals * rep), svals)
    x = x - freq * oc_c
    x = x - pres * out_mask.astype(np.float32)
    x = x / temp_eff
    return x.astype(np.float32)


def _solve_row(svals, sids, x, lp, gmax, lshift, bound, temp, tk, tp,
               gum_row, nlp):
    """Finish one row on its (sorted-desc by raw value) candidates.

    Returns (sampled, indices[nlp+1], lps[nlp+1], rank) or None if the
    candidate set cannot be proven to cover everything the reference math
    looks at (caller falls back to exact host candidates).
    """
    ncand = svals.shape[0]
    finite_bound = np.isfinite(bound)
    if ncand < max(nlp, 64) or not svals[nlp - 1] > bound:
        return None

    temp_eff = np.float32(1.0) if temp < _EPS else temp

    xo = np.lexsort((sids, -x))
    xs = x[xo]
    xids = sids[xo]
    if finite_bound:
        # every non-candidate's penalized value is <= bound/temp (tiny slack
        # for f32 rounding); need the full top-64 of x inside the candidates
        nc_bound = np.float64(bound) / np.float64(temp_eff)
        nc_bound += 1e-5 * abs(nc_bound) + 1e-6
        if not np.float64(xs[63]) > nc_bound:
            return None

    k = int(min(max(int(tk), 1), V))
    kept_x = xs[:k]
    e = np.exp(kept_x - kept_x[0], dtype=np.float32)
    denom = np.sum(e, dtype=np.float32)
    probs = e / denom
    cum = np.cumsum(probs, dtype=np.float32)
    keep = (cum - probs) < tp
    kept_ids = xids[:k][keep]
    kept_vals = kept_x[keep]

    y = kept_vals + gum_row[kept_ids]
    rand_sampled = int(kept_ids[y == y.max()].min())
    greedy = int(xids[0])
    sampled = greedy if temp < _EPS else rand_sampled

    pos = int(np.nonzero(sids == sampled)[0][0])
    lp_s = lp[pos]
    if finite_bound:
        lp_b = np.float32(np.float32(bound - gmax) - lshift)
        if not (np.float64(lp_s) - np.float64(lp_b) > 1e-3):
            return None
    rank = int(np.count_nonzero(lp >= lp_s))

    indices = np.concatenate(([sampled], sids[:nlp])).astype(np.int32)
    lps = np.concatenate(([lp_s], lp[:nlp])).astype(np.float32)
    return sampled, indices, lps, rank


def kernel(logits, temperature, top_k, top_p, presence_penalties,
           frequency_penalties, repetition_penalties, prompt_token_ids,
           output_token_ids, gumbel, num_logprobs):
    logits = np.asarray(logits, dtype=np.float32)
    temperature = np.asarray(temperature, dtype=np.float32)
    top_k = np.asarray(top_k, dtype=np.int32)
    top_p = np.asarray(top_p, dtype=np.float32)
    presence_penalties = np.asarray(presence_penalties, dtype=np.float32)
    frequency_penalties = np.asarray(frequency_penalties, dtype=np.float32)
    repetition_penalties = np.asarray(repetition_penalties, dtype=np.float32)
    prompt_token_ids = np.asarray(prompt_token_ids, dtype=np.int64)
    output_token_ids = np.asarray(output_token_ids, dtype=np.int64)
    gumbel = np.asarray(gumbel, dtype=np.float32)
    nlp = int(num_logprobs)

    vals, ids, bound, sum0, bad = _run_device(logits)

    # penalty bookkeeping (exact integer-valued f32, same as reference)
    rows = np.arange(N)[:, None]
    pm = np.zeros((N, V), np.bool_)
    pm[rows, prompt_token_ids] = True
    oc = np.zeros((N, V), np.uint8)
    np.add.at(oc, (rows, output_token_ids), 1)

    svals, sids = _sort_desc(vals, ids)
    pm_c = np.take_along_axis(pm, sids, axis=1)
    oc_c = np.take_along_axis(oc, sids, axis=1).astype(np.float32)
    x = _penalized(
        svals, pm_c, oc_c,
        repetition_penalties[:, None], frequency_penalties[:, None],
        presence_penalties[:, None],
        np.where(temperature < _EPS, np.float32(1.0), temperature)[:, None],
    )
    gmax = svals[:, 0]
    lshift = (np.log(sum0) - gmax.astype(np.float64)).astype(np.float32)
    lp = ((svals - gmax[:, None]) - lshift[:, None]).astype(np.float32)

    sampled_out = np.zeros(N, np.int32)
    indices_out = np.zeros((N, nlp + 1), np.int32)
    lps_out = np.zeros((N, nlp + 1), np.float32)
    ranks_out = np.zeros(N, np.int32)

    LAST["fallback_rows"] = []
    for r in range(N):
        res = None
        if not bad[r]:
            res = _solve_row(svals[r], sids[r], x[r], lp[r], gmax[r],
                             lshift[r], bound[r], temperature[r], top_k[r],
                             top_p[r], gumbel[r], nlp)
        if res is None:
            LAST["fallback_rows"].append(r)
            res = _fallback_row(logits[r], sum0[r], pm[r], oc[r],
                                repetition_penalties[r], frequency_penalties[r],
                                presence_penalties[r], temperature[r],
                                top_k[r], top_p[r], gumbel[r], nlp)
        sampled_out[r], indices_out[r], lps_out[r], ranks_out[r] = res

    return sampled_out, indices_out, lps_out, ranks_out


def _fallback_row(row_logits, s0, pm_r, oc_r, rep, freq, pres, temp, tk, tp,
                  gum_row, nlp):
    """Exact host path for rows where device candidates can't be verified."""
    for ncand in (CAND, V):
        if ncand < V:
            part = np.argpartition(-row_logits, ncand - 1)[:ncand]
            cvals, cids = row_logits[part], part.astype(np.int64)
            o = np.lexsort((cids, -cvals))
            cvals, cids = cvals[o], cids[o]
            b = np.float32(cvals[-1])
        else:
            cids = np.arange(V, dtype=np.int64)
            o = np.lexsort((cids, -row_logits))
            cvals, cids = row_logits[o].copy(), cids[o]
            b = np.float32(-np.inf)
        temp_eff = np.float32(1.0) if temp < _EPS else temp
        x = _penalized(cvals, pm_r[cids], oc_r[cids].astype(np.float32),
                       np.float32(rep), np.float32(freq), np.float32(pres),
                       temp_eff)
        gmax = cvals[0]
        lshift = np.float32(np.log(s0) - np.float64(gmax))
        lp = ((cvals - gmax) - lshift).astype(np.float32)
        res = _solve_row(cvals, cids, x, lp, gmax, lshift, b, temp, tk, tp,
                         gum_row, nlp)
        if res is not None:
            return res
    raise RuntimeError("fallback failed even with full-vocab candidates")


# revision 14
# speedup vs baseline: 22.8273x; 22.8273x over previous
"""Trainium2 Bass kernel for vLLM-style sampler (nn_Sampler_23897198035285).

Strategy
--------
The reference does, per row (N=256, V=128000):
  raw logprobs (log_softmax of logits), penalty application (<=96 touched
  positions/row), temperature, top-k (k<=63) / top-p filtering on a full
  descending sort, Gumbel-max sampling, and top-20 logprob gathering.

Everything downstream of the full-vocab scan only ever looks at the extreme
top of each row:
  * tokens surviving top-k have penalized-rank < k <= 63;
  * penalties only *decrease* values and touch <= 96 positions, so any token
    in the penalized top-64 has raw rank <= 64+96 = 160;
  * the sampled token's rank among raw logprobs is likewise <= 160.

So the device only needs two streaming quantities per row:
  1. sum(exp(logits)) for the log-softmax normalizer, and
  2. a candidate set provably containing the raw top-~160:
     the top-8 of each 500-element chunk (256 chunks/row -> 2048 candidates,
     with the max over per-chunk 8th-maxes giving an exact upper bound B on
     every non-candidate value).

Rows are sharded across the 8 cores (32 rows/core, zero communication).
Each core streams its 16.4MB logits shard once: DVE max/max_index extracts
candidates, ACT computes exp with a fused per-partition accumulate.
The host then finishes each row exactly on <=2048 candidates, verifying
coverage via B; rows that fail verification (statistically ~never) fall back
to exact host candidate extraction. `gumbel` is only ever read at the <=63
kept positions per row, so it is never shipped to the device at all.
"""

import numpy as np

N, V = 256, 128000
NCORES = 8
RPC = N // NCORES          # rows per core
P = 128                    # SBUF partitions
F = V // P                 # 1000 elements per partition per row
CH = 2                     # chunks per partition
CF = F // CH               # 500-element chunk
K8 = 8                     # top-k extracted per chunk (HW max8 width)
CAND = P * CH * K8         # 2048 candidates per row
_EPS = 1e-5

# test.py pokes these for profiling; the harness never touches them.
TRACE = False
LAST = {}

_prog = None


def _build_program(repeats=1):
    import concourse.mybir as mybir
    import concourse.tile as tile
    from concourse import bacc

    nc = bacc.Bacc("TRN2", target_bir_lowering=False, debug=False,
                   enable_asserts=False)
    f32 = mybir.dt.float32
    logits = nc.dram_tensor("logits", [RPC, V], f32, kind="ExternalInput")
    cand_idx = nc.dram_tensor("cand_idx", [P, RPC * CH * K8], mybir.dt.uint16,
                              kind="ExternalOutput")
    sumexp = nc.dram_tensor("sumexp", [P, RPC], f32, kind="ExternalOutput")

    with tile.TileContext(nc) as tc:
        with (
            tc.tile_pool(name="inp", bufs=6) as inp,
            tc.tile_pool(name="outp", bufs=1) as outp,
        ):
            vals_t = outp.tile([P, RPC * CH * K8], mybir.dt.bfloat16, tag="vals")
            idx_t = outp.tile([P, RPC * CH * K8], mybir.dt.uint16, tag="idx")
            se_t = outp.tile([P, RPC], f32, tag="se")
            # Per-row disjoint bf16 slices for the exp output. exp is
            # monotone, so the DVE top-8 extraction runs on the exp values
            # (the raw tile then has exactly one consumer): every
            # instruction here carries at most one semaphore wait, which is
            # all the ACT/DMA ISA structs have room for.
            e = outp.tile([P, RPC * F], mybir.dt.bfloat16, tag="e")

            lg = logits.ap().rearrange("r (p f) -> r p f", p=P)
            for r in [r for _ in range(repeats) for r in range(RPC)]:
                t = inp.tile([P, F], f32, tag="t")
                nc.sync.dma_start(t[:], lg[r])
                er = e[:, r * F:(r + 1) * F]
                nc.scalar.activation(er, t[:],
                                     mybir.ActivationFunctionType.Exp,
                                     accum_out=se_t[:, r:r + 1])
                for c in range(CH):
                    sl = er[:, c * CF:(c + 1) * CF]
                    base = (r * CH + c) * K8
                    vs = vals_t[:, base:base + K8]
                    nc.vector.max(vs, sl)
                    nc.vector.max_index(idx_t[:, base:base + K8], vs, sl)
            nc.sync.dma_start(cand_idx.ap(), idx_t[:])
            nc.sync.dma_start(sumexp.ap(), se_t[:])
    nc.compile()
    return nc


def _run_device(logits):
    global _prog
    if _prog is None:
        _prog = _build_program()
    from concourse.bass_utils import run_bass_kernel_spmd

    in_maps = [
        {"logits": np.ascontiguousarray(logits[c * RPC:(c + 1) * RPC])}
        for c in range(NCORES)
    ]
    out = run_bass_kernel_spmd(_prog, in_maps, core_ids=list(range(NCORES)),
                               trace=TRACE)
    LAST["exec_time_ns"] = out.exec_time_ns
    rs = out.results

    ci = np.stack([r["cand_idx"] for r in rs])           # [8,128,RPC*16] u16
    se = np.stack([r["sumexp"] for r in rs])             # [8,128,RPC]

    ci = ci.reshape(NCORES, P, RPC, CH * K8).transpose(0, 2, 1, 3)
    ci = ci.reshape(N, P, CH, K8).astype(np.int64)

    ids = (np.arange(P, dtype=np.int64)[None, :, None, None] * F
           + np.arange(CH, dtype=np.int64)[None, None, :, None] * CF
           + np.minimum(ci, CF - 1))

    bad = (ci >= CF).reshape(N, -1).any(axis=1)          # max_index anomaly

    # exact f32 candidate values, gathered from the input itself
    vals = logits[np.arange(N)[:, None], ids.reshape(N, CAND)]
    ids = ids.reshape(N, CAND)

    # The device extracted the per-chunk top-8 *of the bf16 exp image*.
    # Every non-extracted element of a chunk maps to a bf16-exp bucket <=
    # the smallest extracted bucket, so its raw value is below the smallest
    # extracted raw value plus one bucket width (~0.004 in log space) plus
    # exp-table slack; 0.02 is generous for both.
    v4 = vals.reshape(N, P, CH, K8)
    bound = (v4.min(axis=3).max(axis=(1, 2)) + np.float32(0.02)).astype(np.float32)

    sid_sorted = np.sort(ids, axis=1)
    bad |= (sid_sorted[:, 1:] == sid_sorted[:, :-1]).any(axis=1)
    bad |= ~(bound > 0)

    sum0 = se.transpose(0, 2, 1).reshape(N, P).sum(axis=1, dtype=np.float64)
    return vals, ids, bound, sum0, bad


def _sort_desc(vals, ids):
    """Descending by value, ties by ascending id (matches stable argsort(-x))."""
    o = np.lexsort((ids, -vals), axis=-1)
    return (np.take_along_axis(vals, o, axis=-1),
            np.take_along_axis(ids, o, axis=-1))


def _penalized(svals, pm_c, oc_c, rep, freq, pres, temp_eff):
    """f32 mimicry of the reference penalty/temperature math, per candidate.

    svals: candidate raw logits (f32), pm_c: prompt-seen mask, oc_c: output
    counts (f32), penalties broadcastable to svals.
    """
    out_mask = oc_c > 0
    seen = pm_c | out_mask
    x = np.where(seen, np.where(svals > 0, svals / rep, svals * rep), svals)
    x = x - freq * oc_c
    x = x - pres * out_mask.astype(np.float32)
    x = x / temp_eff
    return x.astype(np.float32)


def _solve_row(svals, sids, x, lp, gmax, lshift, bound, temp, tk, tp,
               gum_row, nlp):
    """Finish one row on its (sorted-desc by raw value) candidates.

    Returns (sampled, indices[nlp+1], lps[nlp+1], rank) or None if the
    candidate set cannot be proven to cover everything the reference math
    looks at (caller falls back to exact host candidates).
    """
    ncand = svals.shape[0]
    finite_bound = np.isfinite(bound)
    if ncand < max(nlp, 64) or not svals[nlp - 1] > bound:
        return None

    temp_eff = np.float32(1.0) if temp < _EPS else temp

    xo = np.lexsort((sids, -x))
    xs = x[xo]
    xids = sids[xo]
    if finite_bound:
        # every non-candidate's penalized value is <= bound/temp (tiny slack
        # for f32 rounding); need the full top-64 of x inside the candidates
        nc_bound = np.float64(bound) / np.float64(temp_eff)
        nc_bound += 1e-5 * abs(nc_bound) + 1e-6
        if not np.float64(xs[63]) > nc_bound:
            return None

    k = int(min(max(int(tk), 1), V))
    kept_x = xs[:k]
    e = np.exp(kept_x - kept_x[0], dtype=np.float32)
    denom = np.sum(e, dtype=np.float32)
    probs = e / denom
    cum = np.cumsum(probs, dtype=np.float32)
    keep = (cum - probs) < tp
    kept_ids = xids[:k][keep]
    kept_vals = kept_x[keep]

    y = kept_vals + gum_row[kept_ids]
    rand_sampled = int(kept_ids[y == y.max()].min())
    greedy = int(xids[0])
    sampled = greedy if temp < _EPS else rand_sampled

    pos = int(np.nonzero(sids == sampled)[0][0])
    lp_s = lp[pos]
    if finite_bound:
        lp_b = np.float32(np.float32(bound - gmax) - lshift)
        if not (np.float64(lp_s) - np.float64(lp_b) > 1e-3):
            return None
    rank = int(np.count_nonzero(lp >= lp_s))

    indices = np.concatenate(([sampled], sids[:nlp])).astype(np.int32)
    lps = np.concatenate(([lp_s], lp[:nlp])).astype(np.float32)
    return sampled, indices, lps, rank


def kernel(logits, temperature, top_k, top_p, presence_penalties,
           frequency_penalties, repetition_penalties, prompt_token_ids,
           output_token_ids, gumbel, num_logprobs):
    logits = np.asarray(logits, dtype=np.float32)
    temperature = np.asarray(temperature, dtype=np.float32)
    top_k = np.asarray(top_k, dtype=np.int32)
    top_p = np.asarray(top_p, dtype=np.float32)
    presence_penalties = np.asarray(presence_penalties, dtype=np.float32)
    frequency_penalties = np.asarray(frequency_penalties, dtype=np.float32)
    repetition_penalties = np.asarray(repetition_penalties, dtype=np.float32)
    prompt_token_ids = np.asarray(prompt_token_ids, dtype=np.int64)
    output_token_ids = np.asarray(output_token_ids, dtype=np.int64)
    gumbel = np.asarray(gumbel, dtype=np.float32)
    nlp = int(num_logprobs)

    vals, ids, bound, sum0, bad = _run_device(logits)

    # penalty bookkeeping (exact integer-valued f32, same as reference)
    rows = np.arange(N)[:, None]
    pm = np.zeros((N, V), np.bool_)
    pm[rows, prompt_token_ids] = True
    oc = np.zeros((N, V), np.uint8)
    np.add.at(oc, (rows, output_token_ids), 1)

    svals, sids = _sort_desc(vals, ids)
    pm_c = np.take_along_axis(pm, sids, axis=1)
    oc_c = np.take_along_axis(oc, sids, axis=1).astype(np.float32)
    x = _penalized(
        svals, pm_c, oc_c,
        repetition_penalties[:, None], frequency_penalties[:, None],
        presence_penalties[:, None],
        np.where(temperature < _EPS, np.float32(1.0), temperature)[:, None],
    )
    gmax = svals[:, 0]
    lshift = (np.log(sum0) - gmax.astype(np.float64)).astype(np.float32)
    lp = ((svals - gmax[:, None]) - lshift[:, None]).astype(np.float32)

    sampled_out = np.zeros(N, np.int32)
    indices_out = np.zeros((N, nlp + 1), np.int32)
    lps_out = np.zeros((N, nlp + 1), np.float32)
    ranks_out = np.zeros(N, np.int32)

    LAST["fallback_rows"] = []
    for r in range(N):
        res = None
        if not bad[r]:
            res = _solve_row(svals[r], sids[r], x[r], lp[r], gmax[r],
                             lshift[r], bound[r], temperature[r], top_k[r],
                             top_p[r], gumbel[r], nlp)
        if res is None:
            LAST["fallback_rows"].append(r)
            res = _fallback_row(logits[r], sum0[r], pm[r], oc[r],
                                repetition_penalties[r], frequency_penalties[r],
                                presence_penalties[r], temperature[r],
                                top_k[r], top_p[r], gumbel[r], nlp)
        sampled_out[r], indices_out[r], lps_out[r], ranks_out[r] = res

    return sampled_out, indices_out, lps_out, ranks_out


def _fallback_row(row_logits, s0, pm_r, oc_r, rep, freq, pres, temp, tk, tp,
                  gum_row, nlp):
    """Exact host path for rows where device candidates can't be verified."""
    for ncand in (CAND, V):
        if ncand < V:
            part = np.argpartition(-row_logits, ncand - 1)[:ncand]
            cvals, cids = row_logits[part], part.astype(np.int64)
            o = np.lexsort((cids, -cvals))
            cvals, cids = cvals[o], cids[o]
            b = np.float32(cvals[-1])
        else:
            cids = np.arange(V, dtype=np.int64)
            o = np.lexsort((cids, -row_logits))
            cvals, cids = row_logits[o].copy(), cids[o]
            b = np.float32(-np.inf)
        temp_eff = np.float32(1.0) if temp < _EPS else temp
        x = _penalized(cvals, pm_r[cids], oc_r[cids].astype(np.float32),
                       np.float32(rep), np.float32(freq), np.float32(pres),
                       temp_eff)
        gmax = cvals[0]
        lshift = np.float32(np.log(s0) - np.float64(gmax))
        lp = ((cvals - gmax) - lshift).astype(np.float32)
        res = _solve_row(cvals, cids, x, lp, gmax, lshift, b, temp, tk, tp,
                         gum_row, nlp)
        if res is not None:
            return res
    raise RuntimeError("fallback failed even with full-vocab candidates")


# revision 17
# speedup vs baseline: 69.2046x; 3.0317x over previous
"""Trainium2 Bass kernel for vLLM-style sampler (nn_Sampler_23897198035285).

Strategy
--------
The reference does, per row (N=256, V=128000):
  raw logprobs (log_softmax of logits), penalty application (<=96 touched
  positions/row), temperature, top-k (k<=63) / top-p filtering on a full
  descending sort, Gumbel-max sampling, and top-20 logprob gathering.

Everything downstream of the full-vocab scan only ever looks at the extreme
top of each row:
  * tokens surviving top-k have penalized-rank < k <= 63;
  * penalties only *decrease* values and touch <= 96 positions, so any token
    in the penalized top-64 has raw rank <= 64+96 = 160;
  * the sampled token's rank among raw logprobs is likewise <= 160.

So the device only needs two streaming quantities per row:
  1. sum(exp(logits)) for the log-softmax normalizer, and
  2. a candidate set provably containing the raw top-~160:
     the top-8 of each 500-element chunk (256 chunks/row -> 2048 candidates,
     with the max over per-chunk 8th-maxes giving an exact upper bound B on
     every non-candidate value).

Rows are sharded across the 8 cores (32 rows/core, zero communication).
Each core streams its 16.4MB logits shard once: DVE max/max_index extracts
candidates, ACT computes exp with a fused per-partition accumulate.
The host then finishes each row exactly on <=2048 candidates, verifying
coverage via B; rows that fail verification (statistically ~never) fall back
to exact host candidate extraction. `gumbel` is only ever read at the <=63
kept positions per row, so it is never shipped to the device at all.
"""

import numpy as np

N, V = 256, 128000
NCORES = 8
RPC = N // NCORES          # rows per core
P = 128                    # SBUF partitions
F = V // P                 # 1000 elements per partition per row
CH = 2                     # chunks per partition
CF = F // CH               # 500-element chunk
K8 = 8                     # top-k extracted per chunk (HW max8 width)
CAND = P * CH * K8         # 2048 candidates per row
_EPS = 1e-5

# test.py reads this for diagnostics; the harness never touches it.
LAST = {}

_prog = None


def _build_program(repeats=1):
    import concourse.mybir as mybir
    import concourse.tile as tile
    from concourse import bacc

    nc = bacc.Bacc("TRN2", target_bir_lowering=False, debug=False,
                   enable_asserts=False)
    f32 = mybir.dt.float32
    logits = nc.dram_tensor("logits", [RPC, V], f32, kind="ExternalInput")
    cand_idx = nc.dram_tensor("cand_idx", [P, RPC * CH * K8], mybir.dt.uint16,
                              kind="ExternalOutput")
    sumexp = nc.dram_tensor("sumexp", [P, RPC], f32, kind="ExternalOutput")

    with tile.TileContext(nc) as tc:
        with (
            tc.tile_pool(name="inp", bufs=6) as inp,
            tc.tile_pool(name="outp", bufs=1) as outp,
        ):
            vals_t = outp.tile([P, RPC * CH * K8], mybir.dt.bfloat16, tag="vals")
            idx_t = outp.tile([P, RPC * CH * K8], mybir.dt.uint16, tag="idx")
            se_t = outp.tile([P, RPC], f32, tag="se")
            # Per-row disjoint bf16 slices for the exp output. exp is
            # monotone, so the DVE top-8 extraction runs on the exp values
            # (the raw tile then has exactly one consumer): every
            # instruction here carries at most one semaphore wait, which is
            # all the ACT/DMA ISA structs have room for.
            e = outp.tile([P, RPC * F], mybir.dt.bfloat16, tag="e")

            lg = logits.ap().rearrange("r (p f) -> r p f", p=P)
            for r in [r for _ in range(repeats) for r in range(RPC)]:
                t = inp.tile([P, F], f32, tag="t")
                nc.sync.dma_start(t[:], lg[r])
                er = e[:, r * F:(r + 1) * F]
                nc.scalar.activation(er, t[:],
                                     mybir.ActivationFunctionType.Exp,
                                     accum_out=se_t[:, r:r + 1])
                for c in range(CH):
                    sl = er[:, c * CF:(c + 1) * CF]
                    base = (r * CH + c) * K8
                    vs = vals_t[:, base:base + K8]
                    nc.vector.max(vs, sl)
                    nc.vector.max_index(idx_t[:, base:base + K8], vs, sl)
            nc.sync.dma_start(cand_idx.ap(), idx_t[:])
            nc.sync.dma_start(sumexp.ap(), se_t[:])
    nc.compile()
    return nc


def _run_device(logits):
    global _prog
    if _prog is None:
        _prog = _build_program()
    from concourse.bass_utils import run_bass_kernel_spmd

    in_maps = [
        {"logits": np.ascontiguousarray(logits[c * RPC:(c + 1) * RPC])}
        for c in range(NCORES)
    ]
    out = run_bass_kernel_spmd(_prog, in_maps, core_ids=list(range(NCORES)))
    LAST["exec_time_ns"] = out.exec_time_ns
    rs = out.results

    ci = np.stack([r["cand_idx"] for r in rs])           # [8,128,RPC*16] u16
    se = np.stack([r["sumexp"] for r in rs])             # [8,128,RPC]

    ci = ci.reshape(NCORES, P, RPC, CH * K8).transpose(0, 2, 1, 3)
    ci = ci.reshape(N, P, CH, K8).astype(np.int64)

    ids = (np.arange(P, dtype=np.int64)[None, :, None, None] * F
           + np.arange(CH, dtype=np.int64)[None, None, :, None] * CF
           + np.minimum(ci, CF - 1))

    bad = (ci >= CF).reshape(N, -1).any(axis=1)          # max_index anomaly

    # exact f32 candidate values, gathered from the input itself
    vals = logits[np.arange(N)[:, None], ids.reshape(N, CAND)]
    ids = ids.reshape(N, CAND)

    # The device extracted the per-chunk top-8 *of the bf16 exp image*.
    # Every non-extracted element of a chunk maps to a bf16-exp bucket <=
    # the smallest extracted bucket, so its raw value is below the smallest
    # extracted raw value plus one bucket width (~0.004 in log space) plus
    # exp-table slack; 0.02 is generous for both.
    v4 = vals.reshape(N, P, CH, K8)
    bound = (v4.min(axis=3).max(axis=(1, 2)) + np.float32(0.02)).astype(np.float32)

    sid_sorted = np.sort(ids, axis=1)
    bad |= (sid_sorted[:, 1:] == sid_sorted[:, :-1]).any(axis=1)
    bad |= ~(bound > 0)

    sum0 = se.transpose(0, 2, 1).reshape(N, P).sum(axis=1, dtype=np.float64)
    return vals, ids, bound, sum0, bad


def _sort_desc(vals, ids):
    """Descending by value, ties by ascending id (matches stable argsort(-x))."""
    o = np.lexsort((ids, -vals), axis=-1)
    return (np.take_along_axis(vals, o, axis=-1),
            np.take_along_axis(ids, o, axis=-1))


def _penalized(svals, pm_c, oc_c, rep, freq, pres, temp_eff):
    """f32 mimicry of the reference penalty/temperature math, per candidate.

    svals: candidate raw logits (f32), pm_c: prompt-seen mask, oc_c: output
    counts (f32), penalties broadcastable to svals.
    """
    out_mask = oc_c > 0
    seen = pm_c | out_mask
    x = np.where(seen, np.where(svals > 0, svals / rep, svals * rep), svals)
    x = x - freq * oc_c
    x = x - pres * out_mask.astype(np.float32)
    x = x / temp_eff
    return x.astype(np.float32)


def _solve_row(svals, sids, x, lp, gmax, lshift, bound, temp, tk, tp,
               gum_row, nlp):
    """Finish one row on its (sorted-desc by raw value) candidates.

    Returns (sampled, indices[nlp+1], lps[nlp+1], rank) or None if the
    candidate set cannot be proven to cover everything the reference math
    looks at (caller falls back to exact host candidates).
    """
    ncand = svals.shape[0]
    k = int(min(max(int(tk), 1), V))
    need = max(nlp, k + 1)
    finite_bound = np.isfinite(bound)
    if ncand < need or not svals[nlp - 1] > bound:
        return None

    temp_eff = np.float32(1.0) if temp < _EPS else temp

    xo = np.lexsort((sids, -x))
    xs = x[xo]
    xids = sids[xo]
    if finite_bound:
        # every non-candidate's penalized value is <= bound/temp (tiny slack
        # for f32 rounding); need the top-(k+1) of x inside the candidates
        nc_bound = np.float64(bound) / np.float64(temp_eff)
        nc_bound += 1e-5 * abs(nc_bound) + 1e-6
        if not np.float64(xs[need - 1]) > nc_bound:
            return None
    kept_x = xs[:k]
    e = np.exp(kept_x - kept_x[0], dtype=np.float32)
    denom = np.sum(e, dtype=np.float32)
    probs = e / denom
    cum = np.cumsum(probs, dtype=np.float32)
    keep = (cum - probs) < tp
    kept_ids = xids[:k][keep]
    kept_vals = kept_x[keep]

    y = kept_vals + gum_row[kept_ids]
    rand_sampled = int(kept_ids[y == y.max()].min())
    greedy = int(xids[0])
    sampled = greedy if temp < _EPS else rand_sampled

    pos = int(np.nonzero(sids == sampled)[0][0])
    lp_s = lp[pos]
    if finite_bound:
        lp_b = np.float32(np.float32(bound - gmax) - lshift)
        if not (np.float64(lp_s) - np.float64(lp_b) > 1e-3):
            return None
    rank = int(np.count_nonzero(lp >= lp_s))

    indices = np.concatenate(([sampled], sids[:nlp])).astype(np.int32)
    lps = np.concatenate(([lp_s], lp[:nlp])).astype(np.float32)
    return sampled, indices, lps, rank


def kernel(logits, temperature, top_k, top_p, presence_penalties,
           frequency_penalties, repetition_penalties, prompt_token_ids,
           output_token_ids, gumbel, num_logprobs):
    logits = np.asarray(logits, dtype=np.float32)
    temperature = np.asarray(temperature, dtype=np.float32)
    top_k = np.asarray(top_k, dtype=np.int32)
    top_p = np.asarray(top_p, dtype=np.float32)
    presence_penalties = np.asarray(presence_penalties, dtype=np.float32)
    frequency_penalties = np.asarray(frequency_penalties, dtype=np.float32)
    repetition_penalties = np.asarray(repetition_penalties, dtype=np.float32)
    prompt_token_ids = np.asarray(prompt_token_ids, dtype=np.int64)
    output_token_ids = np.asarray(output_token_ids, dtype=np.int64)
    gumbel = np.asarray(gumbel, dtype=np.float32)
    nlp = int(num_logprobs)

    vals, ids, bound, sum0, bad = _run_device(logits)

    # penalty bookkeeping (exact integer-valued f32, same as reference)
    rows = np.arange(N)[:, None]
    pm = np.zeros((N, V), np.bool_)
    pm[rows, prompt_token_ids] = True
    oc = np.zeros((N, V), np.uint8)
    np.add.at(oc, (rows, output_token_ids), 1)

    svals, sids = _sort_desc(vals, ids)
    pm_c = np.take_along_axis(pm, sids, axis=1)
    oc_c = np.take_along_axis(oc, sids, axis=1).astype(np.float32)
    x = _penalized(
        svals, pm_c, oc_c,
        repetition_penalties[:, None], frequency_penalties[:, None],
        presence_penalties[:, None],
        np.where(temperature < _EPS, np.float32(1.0), temperature)[:, None],
    )
    gmax = svals[:, 0]
    lshift = (np.log(sum0) - gmax.astype(np.float64)).astype(np.float32)
    lp = ((svals - gmax[:, None]) - lshift[:, None]).astype(np.float32)

    sampled_out = np.zeros(N, np.int32)
    indices_out = np.zeros((N, nlp + 1), np.int32)
    lps_out = np.zeros((N, nlp + 1), np.float32)
    ranks_out = np.zeros(N, np.int32)

    LAST["fallback_rows"] = []
    for r in range(N):
        res = None
        if not bad[r]:
            res = _solve_row(svals[r], sids[r], x[r], lp[r], gmax[r],
                             lshift[r], bound[r], temperature[r], top_k[r],
                             top_p[r], gumbel[r], nlp)
        if res is None:
            LAST["fallback_rows"].append(r)
            res = _fallback_row(logits[r], sum0[r], pm[r], oc[r],
                                repetition_penalties[r], frequency_penalties[r],
                                presence_penalties[r], temperature[r],
                                top_k[r], top_p[r], gumbel[r], nlp)
        sampled_out[r], indices_out[r], lps_out[r], ranks_out[r] = res

    return sampled_out, indices_out, lps_out, ranks_out


def _fallback_row(row_logits, s0, pm_r, oc_r, rep, freq, pres, temp, tk, tp,
                  gum_row, nlp):
    """Exact host path for rows where device candidates can't be verified."""
    for ncand in (CAND, V):
        if ncand < V:
            part = np.argpartition(-row_logits, ncand - 1)[:ncand]
            cvals, cids = row_logits[part], part.astype(np.int64)
            o = np.lexsort((cids, -cvals))
            cvals, cids = cvals[o], cids[o]
            b = np.float32(cvals[-1])
        else:
            cids = np.arange(V, dtype=np.int64)
            o = np.lexsort((cids, -row_logits))
            cvals, cids = row_logits[o].copy(), cids[o]
            b = np.float32(-np.inf)
        temp_eff = np.float32(1.0) if temp < _EPS else temp
        x = _penalized(cvals, pm_r[cids], oc_r[cids].astype(np.float32),
                       np.float32(rep), np.float32(freq), np.float32(pres),
                       temp_eff)
        gmax = cvals[0]
        lshift = np.float32(np.log(s0) - np.float64(gmax))
        lp = ((cvals - gmax) - lshift).astype(np.float32)
        res = _solve_row(cvals, cids, x, lp, gmax, lshift, b, temp, tk, tp,
                         gum_row, nlp)
        if res is not None:
            return res
    raise RuntimeError("fallback failed even with full-vocab candidates")
